# revision 26
# baseline (speedup 1.0000x reference)
"""Trainium2 Bass kernel for nn_BinaryConv2d (B=16, C=64, H=W=256, 3x3, pad 1).

Forward semantics (STE forward values):
  act = sign(x * rd_k + rd_b)                  in {-1, 0, +1}
  bw  = scaling[co] * sign(conv_w)             scaling = mean |conv_w| per out-ch
  y   = prelu(conv2d(act, bw, pad=1) + pr_bias0) + pr_bias1 + x

Strategy: data-parallel over batch, 2 images per core (8 cores).  The two
images' 64 channels are stacked on the 128 SBUF partitions.  x and y travel
through HBM as fp16 (halves DMA vs fp32; fp16 keeps 11-bit mantissa so the
residual path stays accurate).  Activations are binarized to fp8 +-1 on the
Scalar engine; the 3x3 conv is exact integer arithmetic in fp32 PSUM via
fp8 DoubleRow matmuls.  Per 2-row psum pair: 4 DR matmuls (kw0/kw1 pairs per
kh via a 1-elem delta step, plus the kh0/kh1 pair at kw2) + 1 plain matmul
(kh2,kw2) -- 5 streams instead of the naive 9.  Post-ops: with the graded
zero biases, prelu folds to m = max(ps, slope*ps) (one DVE op from PSUM) and
y = scaling*m + x (one Pool op), keeping ACT free for the sign.
"""

import sys

if "/opt/trn_rl_repo" not in sys.path:
    sys.path.insert(0, "/opt/trn_rl_repo")

from contextlib import ExitStack

import ml_dtypes
import numpy as np

import concourse.bacc as bacc
import concourse.bass as bass
import concourse.tile as tile
from concourse import mybir
from concourse.ap import AP
from concourse.bass_utils import run_bass_kernel_spmd

B, C, H, W = 16, 64, 256, 256
NCORES = 8
P = 128                      # partitions = 2 images x 64 channels

F32 = mybir.dt.float32
FP16 = mybir.dt.float16
FP8 = mybir.dt.float8e4
AF = mybir.ActivationFunctionType
ALU = mybir.AluOpType

APITCH = 272                 # act row pitch (16B-aligned for DR row deltas)

# Param table columns (per-partition f32 scalars).  PA=(1-slope)*s,
# PBB=(1-slope)*b0, PD=slope*s, PE=slope*b0+b1 serve the fused
# prelu(v) = slope*v + relu((1-slope)*v) decomposition (valid slope<=1);
# PS/PB0/PCM/PB1 serve the general fallback.
PK, PB, PA, PBB, PD, PEc, PS, PB0, PCM, PB1 = range(10)
NPAR = 12

STRIP_HS = [36] * 6 + [24, 16]   # strip heights (sum == H); small last strip
GROUP_ROWS = 4               # output rows per PSUM group (2 banks)

# 'dr5': 5 matmuls/pair (4 DR N=512 + 1 plain N=512): fewest PE cycles, but
#   the DR-heavy mix plus full engine activity trips the chip power limit and
#   the whole chip downclocks ~18%.
# 'base9': 6 DR N=256 + 3 plain N=512 per pair: more cycles at full clock.
MM_SCHEME = "mix"
MIX_NUM, MIX_DEN = 7, 8      # fraction of pairs using dr5


def _emit(tc, nc, x_d, w_d, p_d, y_d, zero_bias, with_ec):
    x3 = x_d.rearrange("p (h w) -> p h w", w=W)
    y3 = y_d.rearrange("p (h w) -> p h w", w=W)

    with ExitStack() as ctx:
        consts = ctx.enter_context(tc.tile_pool(name="consts", bufs=1))
        xpool = ctx.enter_context(tc.tile_pool(name="xpool", bufs=2))
        apool = ctx.enter_context(tc.tile_pool(name="apool", bufs=2))
        ypool = ctx.enter_context(tc.tile_pool(name="ypool", bufs=2))
        tpool = ctx.enter_context(tc.tile_pool(name="tpool", bufs=4))
        pspool = ctx.enter_context(tc.tile_pool(name="pspool", bufs=4,
                                                space="PSUM"))

        # params first on the load ring (sign needs them); weights on the
        # store ring, which is idle at kernel start
        pt = consts.tile([P, NPAR], F32)
        nc.sync.dma_start(out=pt, in_=p_d)
        # weight slots: [2j+d] = (kh=j, kw=d in {0,1}); [6+d] = (kh=d, kw=2);
        # [8] = (kh=2, kw=2)
        wt = consts.tile([P, 9, 128], FP8)
        nc.scalar.dma_start(out=wt,
                            in_=w_d.rearrange("p (j m) -> p j m", j=9))

        H0S = [sum(STRIP_HS[:i]) for i in range(len(STRIP_HS))]
        NST = len(STRIP_HS)
        HSMAX = max(STRIP_HS)

        def strip_rows(s):
            h0 = H0S[s]
            row_lo = max(h0 - 1, 0)
            row_hi = min(h0 + STRIP_HS[s] + 1, H)
            return h0, row_lo, row_hi, row_lo - (h0 - 1)

        def load_strip(s):
            """DMA the x strip (rows h0-1 .. h0+hs; tile row a <-> global
            h0-1+a) and memset the act padding."""
            h0, row_lo, row_hi, r0 = strip_rows(s)
            nr = row_hi - row_lo
            xs = xpool.tile([P, HSMAX + 2, W], FP16, name="xs")
            if s == 0:                   # tiny first piece: fast start
                bounds = [0, 3, 9, 18, 27, row_hi]
            else:
                bounds = [row_lo, (row_lo + row_hi) // 2, row_hi]
            for a, b in zip(bounds, bounds[1:]):
                if b > a:
                    nc.sync.dma_start(out=xs[:, a - (h0 - 1):b - (h0 - 1), :],
                                      in_=x3[:, a:b, :])
            act = apool.tile([P, HSMAX + 2, APITCH], FP8, name="act")
            nrows = STRIP_HS[s] + 2
            nc.gpsimd.memset(act[:, :nrows, 0:1], 0.0)
            nc.gpsimd.memset(act[:, :nrows, W + 1:W + 2], 0.0)
            if s == 0:
                nc.gpsimd.memset(act[:, 0:1, :], 0.0)
            if s == NST - 1:
                nc.gpsimd.memset(act[:, nrows - 1:nrows, :], 0.0)
            return xs, act

        def sign_strip(s, xs, act, chunks, skip=0):
            """Binarize x into the zero-padded act tile, in row chunks (the
            first small so dependent matmuls unblock quickly)."""
            _, row_lo, row_hi, r0 = strip_rows(s)
            c0 = r0 + skip
            for sz in chunks:
                c1 = min(c0 + sz, r0 + (row_hi - row_lo))
                if c1 <= c0:
                    break
                nc.scalar.activation(
                    act[:, c0:c1, 1:W + 1], xs[:, c0:c1, :], AF.Sign,
                    bias=pt[:, PB:PB + 1], scale=pt[:, PK:PK + 1],
                )
                c0 = c1

        FIRST_CHUNKS = (3, 2, 4, 6, 7, 7, 8)   # strip 0: progressive chunks
        NEXT_CHUNKS = (6, 9, 9, 14)

        ppitch = (HSMAX + 2) * APITCH
        wbase = wt[:, 0, 0]
        woff0 = wbase.offset

        def conv_pair_dr5(ps2, act, r):
            """5 matmuls accumulating the 3x3 binary conv for output rows
            (r, r+1) of the strip into a 2-row psum slice.  act row index
            r+kh+i corresponds to input row (h0-1) + r+kh+i."""
            base = act[:, 0, 0]          # anchor AP for offset math
            off0 = base.offset

            def rhs(row, col, dstep):
                return AP(base.tensor, off0 + row * APITCH + col,
                          [[ppitch, P], [dstep, 2], [APITCH, 2], [1, 256]])

            for j in range(3):           # (kh=j, kw=0)+(kh=j, kw=1) pairs
                nc.tensor.matmul(
                    ps2, lhsT=wt[:, 2 * j:2 * j + 2, :], rhs=rhs(r + j, 0, 1),
                    start=(j == 0), stop=False,
                    perf_mode=mybir.MatmulPerfMode.DoubleRow,
                )
            nc.tensor.matmul(             # (kh=0, kw=2)+(kh=1, kw=2) pair
                ps2, lhsT=wt[:, 6:8, :], rhs=rhs(r, 2, APITCH),
                start=False, stop=False,
                perf_mode=mybir.MatmulPerfMode.DoubleRow,
            )
            nc.tensor.matmul(             # (kh=2, kw=2) plain
                ps2, lhsT=wt[:, 8, :], rhs=act[:, r + 2:r + 4, 2:2 + W],
                start=False, stop=True,
            )

        def conv_pair_base9(ps2, act, r):
            """Baseline scheme: per output row, 3 DR matmuls (kh0+kh1 per kw,
            N=256); then kh2 via 3 plain N=512 matmuls shared by both rows."""
            base = act[:, 0, 0]
            off0 = base.offset

            def wpair(slot_a, stride):
                return AP(wbase.tensor, woff0 + slot_a * 128,
                          [[9 * 128, P], [stride, 2], [1, 128]])

            for i in range(2):           # output row r+i
                po = ps2[:, i, :]
                for kw in range(2):      # (kh0,kh1) DR pairs at kw0/kw1
                    nc.tensor.matmul(
                        po, lhsT=wpair(kw, 256),
                        rhs=AP(base.tensor, off0 + (r + i) * APITCH + kw,
                               [[ppitch, P], [APITCH, 2], [1, 256]]),
                        start=(i == 0 and kw == 0), stop=False,
                        perf_mode=mybir.MatmulPerfMode.DoubleRow,
                    )
                nc.tensor.matmul(        # (kh0,kh1) DR pair at kw2
                    po, lhsT=wt[:, 6:8, :],
                    rhs=AP(base.tensor, off0 + (r + i) * APITCH + 2,
                           [[ppitch, P], [APITCH, 2], [1, 256]]),
                    start=False, stop=False,
                    perf_mode=mybir.MatmulPerfMode.DoubleRow,
                )
            for kw in range(3):          # kh2 plain, both rows
                nc.tensor.matmul(
                    ps2, lhsT=wt[:, (4, 5, 8)[kw], :],
                    rhs=act[:, r + 2:r + 4, kw:kw + W],
                    start=False, stop=(kw == 2),
                )

        if MM_SCHEME == "dr5":
            conv_pair = conv_pair_dr5
        elif MM_SCHEME == "base9":
            conv_pair = conv_pair_base9
        else:                      # 'mix': interleave schemes to sit just
            # under the chip power threshold (dr5 alone trips an ~18%
            # whole-chip downclock; base9 alone wastes PE cycles)
            def conv_pair(ps2, act, r):
                if (r // 2) % MIX_DEN < MIX_NUM:
                    conv_pair_dr5(ps2, act, r)
                else:
                    conv_pair_base9(ps2, act, r)

        SPLIT_FIRST_CHUNK = True
        cur = load_strip(0)
        sign_strip(0, *cur, FIRST_CHUNKS)
        nxt = None
        for s in range(NST):
            h0 = H0S[s]
            HS_S = STRIP_HS[s]
            NG = HS_S // GROUP_ROWS
            xs, act = cur
            ys = ypool.tile([P, HSMAX, W], FP16, name="ys")
            for g in range(NG):
                if g == min(1, NG - 1) and s + 1 < NST:
                    nxt = load_strip(s + 1)   # loads overlap this strip
                if s + 1 < NST:
                    # spread the next strip's sign chunks across this strip's
                    # groups so ACT alternates relu/sign and the next strip's
                    # matmuls never wait at the boundary
                    for ci in range(len(NEXT_CHUNKS)):
                        if g == min(2 * (ci + 1), NG - 1) and g > 1:
                            sign_strip(s + 1, *nxt, NEXT_CHUNKS[ci:ci + 1],
                                       skip=sum(NEXT_CHUNKS[:ci]))
                r = g * GROUP_ROWS
                ps4 = pspool.tile([P, GROUP_ROWS, 256], F32, name="ps")
                conv_pair(ps4[:, 0:2, :], act, r)
                conv_pair(ps4[:, 2:4, :], act, r + 2)
                u4 = ys[:, r:r + GROUP_ROWS, :]
                x4 = xs[:, r + 1:r + 1 + GROUP_ROWS, :]
                if zero_bias:
                    # r = relu((1-slope)*(s*ps + b0)) on ACT;
                    # m = slope*s*ps + r on DVE (single PSUM read);
                    # y = m + x on DVE (all-fp16 SBUF: fast mode)
                    r4 = tpool.tile([P, GROUP_ROWS, W], FP16, name="r")
                    nc.scalar.activation(
                        r4, ps4, AF.Relu,
                        bias=pt[:, PBB:PBB + 1], scale=pt[:, PA:PA + 1],
                    )
                    m4 = tpool.tile([P, GROUP_ROWS, W], FP16, name="m")
                    nc.vector.scalar_tensor_tensor(
                        m4, ps4, pt[:, PD:PD + 1], r4, ALU.mult, ALU.add
                    )
                    nc.vector.tensor_tensor(u4, m4, x4, ALU.add)
                    if with_ec:
                        nc.gpsimd.tensor_scalar(u4, u4, pt[:, PEc:PEc + 1],
                                                None, ALU.add)
                else:
                    # v = ps*s + b0; m = min(v,0)*(slope-1); u = v + m;
                    # y = (u + b1) + x
                    v4 = tpool.tile([P, GROUP_ROWS, W], F32, name="v")
                    nc.vector.tensor_scalar(
                        v4, ps4, pt[:, PS:PS + 1], pt[:, PB0:PB0 + 1],
                        ALU.mult, ALU.add,
                    )
                    m4 = tpool.tile([P, GROUP_ROWS, W], F32, name="mw")
                    nc.vector.tensor_scalar(
                        m4, v4, 0.0, pt[:, PCM:PCM + 1], ALU.min, ALU.mult
                    )
                    nc.vector.tensor_tensor(v4, v4, m4, ALU.add)
                    nc.vector.scalar_tensor_tensor(
                        u4, v4, pt[:, PB1:PB1 + 1], x4, ALU.add, ALU.add
                    )
                # eager stores every 2 groups; alternate HWDGE rings per
                # strip so descriptor gen doesn't pile on one sequencer
                seng = nc.scalar if s % 2 == 0 else nc.sync
                if g % 2 == 1:
                    r0s = (g - 1) * GROUP_ROWS
                    seng.dma_start(
                        out=y3[:, h0 + r0s:h0 + r + GROUP_ROWS, :],
                        in_=ys[:, r0s:r + GROUP_ROWS, :])
            if NG % 2 == 1:              # leftover rows of an odd group count
                r0s = (NG - 1) * GROUP_ROWS
                seng.dma_start(out=y3[:, h0 + r0s:h0 + HS_S, :],
                               in_=ys[:, r0s:HS_S, :])
            cur = nxt


def build_nc(zero_bias=True, with_ec=False):
    nc = bacc.Bacc("TRN2", target_bir_lowering=False, debug=False,
                   num_devices=NCORES)
    x_d = nc.dram_tensor("xin", [P, H * W], FP16, kind="ExternalInput").ap()
    w_d = nc.dram_tensor("wp", [P, 9 * 128], FP8, kind="ExternalInput").ap()
    p_d = nc.dram_tensor("pp", [P, NPAR], F32, kind="ExternalInput").ap()
    y_d = nc.dram_tensor("yout", [P, H * W], FP16, kind="ExternalOutput").ap()
    with tile.TileContext(nc) as tc:
        _emit(tc, nc, x_d, w_d, p_d, y_d, zero_bias, with_ec)
    nc.compile()
    return nc


_NC_CACHE = {}


def _get_nc(zero_bias, with_ec=False):
    key = (zero_bias, with_ec)
    if key not in _NC_CACHE:
        _NC_CACHE[key] = build_nc(zero_bias, with_ec)
    return _NC_CACHE[key]


def make_inputs(x, rd_k, rd_b, beta, conv_w, pr_bias0, prelu_w, pr_bias1):
    """Host-side prep: per-channel param table, packed sign weights, shards."""
    k = np.asarray(rd_k, np.float32).reshape(C)
    b = np.asarray(rd_b, np.float32).reshape(C)
    s = np.mean(np.abs(np.asarray(conv_w, np.float32)), axis=(1, 2, 3))
    b0 = np.asarray(pr_bias0, np.float32).reshape(C)
    slope = np.asarray(prelu_w, np.float32).reshape(C)
    b1 = np.asarray(pr_bias1, np.float32).reshape(C)
    cm = slope - 1.0
    cols = np.stack([
        k, b,
        (1.0 - slope) * s, (1.0 - slope) * b0,          # PA, PBB
        slope * s, slope * b0 + b1,                     # PD, PEc
        s, b0, cm, b1,                                  # fallback
        np.zeros(C, np.float32), np.zeros(C, np.float32),
    ], axis=1)
    pp = np.concatenate([cols, cols], axis=0).astype(np.float32)  # [128, 12]

    sw = np.sign(np.asarray(conv_w, np.float32))  # [co, ci, kh, kw]

    def blockdiag(kh, kw):
        S = sw[:, :, kh, kw].T  # [ci, co]
        out = np.zeros((P, P), np.float32)
        out[0:C, 0:C] = S
        out[C:P, C:P] = S
        return out

    wp = np.zeros((P, 9, 128), np.float32)
    for j in range(3):
        for d in range(2):
            wp[:, 2 * j + d, :] = blockdiag(j, d)
    for d in range(2):
        wp[:, 6 + d, :] = blockdiag(d, 2)
    wp[:, 8, :] = blockdiag(2, 2)
    wp = np.ascontiguousarray(wp.reshape(P, 9 * 128)).astype(mybir.dt.np(FP8))

    x = np.asarray(x, np.float32)
    in_maps = []
    for c in range(NCORES):
        xc = np.ascontiguousarray(x[2 * c:2 * c + 2]).reshape(P, H * W)
        in_maps.append({"xin": xc.astype(np.float16), "wp": wp, "pp": pp})
    return in_maps


def kernel(x, rd_k, rd_b, beta, conv_w, pr_bias0, prelu_w, pr_bias1):
    slope = np.asarray(prelu_w, np.float32).reshape(C)
    # relu decomposition of prelu needs (1-slope) >= 0
    zero_bias = bool(np.all((slope >= 0.0) & (slope <= 1.0)))
    ec = (slope * np.asarray(pr_bias0, np.float32).reshape(C)
          + np.asarray(pr_bias1, np.float32).reshape(C))
    with_ec = bool(np.any(ec != 0.0))
    in_maps = make_inputs(x, rd_k, rd_b, beta, conv_w, pr_bias0, prelu_w,
                          pr_bias1)
    nc = _get_nc(zero_bias, with_ec)
    res = run_bass_kernel_spmd(nc, in_maps, core_ids=list(range(NCORES)))
    y = np.empty((B, C, H, W), np.float32)
    for c in range(NCORES):
        y[2 * c:2 * c + 2] = (
            res.results[c]["yout"].astype(np.float32).reshape(2, C, H, W))
    return y


# revision 27
# speedup vs baseline: 1.1296x; 1.1296x over previous
"""Trainium2 Bass kernel for nn_BinaryConv2d (B=16, C=64, H=W=256, 3x3, pad 1).

Forward semantics (STE forward values):
  act = sign(x * rd_k + rd_b)                  in {-1, 0, +1}
  bw  = scaling[co] * sign(conv_w)             scaling = mean |conv_w| per out-ch
  y   = prelu(conv2d(act, bw, pad=1) + pr_bias0) + pr_bias1 + x

Strategy: data-parallel over batch, 2 images per core (8 cores).  The two
images' 64 channels are stacked on the 128 SBUF partitions.  x and y travel
through HBM as fp16 (halves DMA vs fp32; fp16 keeps 11-bit mantissa so the
residual path stays accurate).  Activations are binarized to fp8 +-1 on the
Scalar engine; the 3x3 conv is exact integer arithmetic in fp32 PSUM via
fp8 DoubleRow matmuls.  Per 2-row psum pair: 4 DR matmuls (kw0/kw1 pairs per
kh via a 1-elem delta step, plus the kh0/kh1 pair at kw2) + 1 plain matmul
(kh2,kw2) -- 5 streams instead of the naive 9.  Post-ops: with the graded
zero biases, prelu folds to m = max(ps, slope*ps) (one DVE op from PSUM) and
y = scaling*m + x (one Pool op), keeping ACT free for the sign.
"""

import sys

if "/opt/trn_rl_repo" not in sys.path:
    sys.path.insert(0, "/opt/trn_rl_repo")

from contextlib import ExitStack

import ml_dtypes
import numpy as np

import concourse.bacc as bacc
import concourse.bass as bass
import concourse.tile as tile
from concourse import mybir
from concourse.ap import AP
from concourse.bass_utils import run_bass_kernel_spmd

B, C, H, W = 16, 64, 256, 256
NCORES = 8
P = 128                      # partitions = 2 images x 64 channels

F32 = mybir.dt.float32
FP16 = mybir.dt.float16
FP8 = mybir.dt.float8e4
AF = mybir.ActivationFunctionType
ALU = mybir.AluOpType

APITCH = 272                 # act row pitch (16B-aligned for DR row deltas)

# Param table columns (per-partition f32 scalars).  PA=(1-slope)*s,
# PBB=(1-slope)*b0, PD=slope*s, PE=slope*b0+b1 serve the fused
# prelu(v) = slope*v + relu((1-slope)*v) decomposition (valid slope<=1);
# PS/PB0/PCM/PB1 serve the general fallback.
PK, PB, PA, PBB, PD, PEc, PS, PB0, PCM, PB1 = range(10)
NPAR = 12

STRIP_HS = [36] * 6 + [24, 16]   # strip heights (sum == H); small last strip
GROUP_ROWS = 4               # output rows per PSUM group (2 banks)

# 'dr5': 5 matmuls/pair (4 DR N=512 + 1 plain N=512): fewest PE cycles, but
#   the DR-heavy mix plus full engine activity trips the chip power limit and
#   the whole chip downclocks ~18%.
# 'base9': 6 DR N=256 + 3 plain N=512 per pair: more cycles at full clock.
MM_SCHEME = "mix"
MIX_NUM, MIX_DEN = 7, 8      # fraction of pairs using dr5


def _emit(tc, nc, x_d, w_d, p_d, y_d, zero_bias, with_ec):
    x3 = x_d.rearrange("p (h w) -> p h w", w=W)
    y3 = y_d.rearrange("p (h w) -> p h w", w=W)

    with ExitStack() as ctx:
        consts = ctx.enter_context(tc.tile_pool(name="consts", bufs=1))
        xpool = ctx.enter_context(tc.tile_pool(name="xpool", bufs=2))
        apool = ctx.enter_context(tc.tile_pool(name="apool", bufs=2))
        ypool = ctx.enter_context(tc.tile_pool(name="ypool", bufs=2))
        tpool = ctx.enter_context(tc.tile_pool(name="tpool", bufs=4))
        pspool = ctx.enter_context(tc.tile_pool(name="pspool", bufs=4,
                                                space="PSUM"))

        # params first on the load ring (sign needs them); weights on the
        # store ring, which is idle at kernel start
        pt = consts.tile([P, NPAR], F32)
        nc.sync.dma_start(out=pt, in_=p_d)
        # weight slots: [2j+d] = (kh=j, kw=d in {0,1}); [6+d] = (kh=d, kw=2);
        # [8] = (kh=2, kw=2)
        wt = consts.tile([P, 9, 128], FP8)
        nc.scalar.dma_start(out=wt,
                            in_=w_d.rearrange("p (j m) -> p j m", j=9))

        H0S = [sum(STRIP_HS[:i]) for i in range(len(STRIP_HS))]
        NST = len(STRIP_HS)
        HSMAX = max(STRIP_HS)

        def strip_rows(s):
            h0 = H0S[s]
            row_lo = max(h0 - 1, 0)
            row_hi = min(h0 + STRIP_HS[s] + 1, H)
            return h0, row_lo, row_hi, row_lo - (h0 - 1)

        def load_strip(s):
            """DMA the x strip (rows h0-1 .. h0+hs; tile row a <-> global
            h0-1+a) and memset the act padding."""
            h0, row_lo, row_hi, r0 = strip_rows(s)
            nr = row_hi - row_lo
            xs = xpool.tile([P, HSMAX + 2, W], FP16, name="xs")
            if s == 0:                   # tiny first piece: fast start
                bounds = [0, 3, 9, 18, 27, row_hi]
            else:
                bounds = [row_lo, (row_lo + row_hi) // 2, row_hi]
            for a, b in zip(bounds, bounds[1:]):
                if b > a:
                    nc.sync.dma_start(out=xs[:, a - (h0 - 1):b - (h0 - 1), :],
                                      in_=x3[:, a:b, :])
            act = apool.tile([P, HSMAX + 2, APITCH], FP8, name="act")
            nrows = STRIP_HS[s] + 2
            nc.gpsimd.memset(act[:, :nrows, 0:1], 0.0)
            nc.gpsimd.memset(act[:, :nrows, W + 1:W + 2], 0.0)
            if s == 0:
                nc.gpsimd.memset(act[:, 0:1, :], 0.0)
            if s == NST - 1:
                nc.gpsimd.memset(act[:, nrows - 1:nrows, :], 0.0)
            return xs, act

        def sign_strip(s, xs, act, chunks, skip=0):
            """Binarize x into the zero-padded act tile, in row chunks (the
            first small so dependent matmuls unblock quickly)."""
            _, row_lo, row_hi, r0 = strip_rows(s)
            c0 = r0 + skip
            for sz in chunks:
                c1 = min(c0 + sz, r0 + (row_hi - row_lo))
                if c1 <= c0:
                    break
                nc.scalar.activation(
                    act[:, c0:c1, 1:W + 1], xs[:, c0:c1, :], AF.Sign,
                    bias=pt[:, PB:PB + 1], scale=pt[:, PK:PK + 1],
                )
                c0 = c1

        FIRST_CHUNKS = (3, 2, 4, 6, 7, 7, 8)   # strip 0: progressive chunks
        NEXT_CHUNKS = (6, 9, 9, 14)

        ppitch = (HSMAX + 2) * APITCH
        wbase = wt[:, 0, 0]
        woff0 = wbase.offset

        def conv_pair_dr5(ps2, act, r):
            """5 matmuls accumulating the 3x3 binary conv for output rows
            (r, r+1) of the strip into a 2-row psum slice.  act row index
            r+kh+i corresponds to input row (h0-1) + r+kh+i."""
            base = act[:, 0, 0]          # anchor AP for offset math
            off0 = base.offset

            def rhs(row, col, dstep):
                return AP(base.tensor, off0 + row * APITCH + col,
                          [[ppitch, P], [dstep, 2], [APITCH, 2], [1, 256]])

            for j in range(3):           # (kh=j, kw=0)+(kh=j, kw=1) pairs
                nc.tensor.matmul(
                    ps2, lhsT=wt[:, 2 * j:2 * j + 2, :], rhs=rhs(r + j, 0, 1),
                    start=(j == 0), stop=False,
                    perf_mode=mybir.MatmulPerfMode.DoubleRow,
                )
            nc.tensor.matmul(             # (kh=0, kw=2)+(kh=1, kw=2) pair
                ps2, lhsT=wt[:, 6:8, :], rhs=rhs(r, 2, APITCH),
                start=False, stop=False,
                perf_mode=mybir.MatmulPerfMode.DoubleRow,
            )
            nc.tensor.matmul(             # (kh=2, kw=2) plain
                ps2, lhsT=wt[:, 8, :], rhs=act[:, r + 2:r + 4, 2:2 + W],
                start=False, stop=True,
            )

        def conv_pair_base9(ps2, act, r):
            """Baseline scheme: per output row, 3 DR matmuls (kh0+kh1 per kw,
            N=256); then kh2 via 3 plain N=512 matmuls shared by both rows."""
            base = act[:, 0, 0]
            off0 = base.offset

            def wpair(slot_a, stride):
                return AP(wbase.tensor, woff0 + slot_a * 128,
                          [[9 * 128, P], [stride, 2], [1, 128]])

            for i in range(2):           # output row r+i
                po = ps2[:, i, :]
                for kw in range(2):      # (kh0,kh1) DR pairs at kw0/kw1
                    nc.tensor.matmul(
                        po, lhsT=wpair(kw, 256),
                        rhs=AP(base.tensor, off0 + (r + i) * APITCH + kw,
                               [[ppitch, P], [APITCH, 2], [1, 256]]),
                        start=(i == 0 and kw == 0), stop=False,
                        perf_mode=mybir.MatmulPerfMode.DoubleRow,
                    )
                nc.tensor.matmul(        # (kh0,kh1) DR pair at kw2
                    po, lhsT=wt[:, 6:8, :],
                    rhs=AP(base.tensor, off0 + (r + i) * APITCH + 2,
                           [[ppitch, P], [APITCH, 2], [1, 256]]),
                    start=False, stop=False,
                    perf_mode=mybir.MatmulPerfMode.DoubleRow,
                )
            for kw in range(3):          # kh2 plain, both rows
                nc.tensor.matmul(
                    ps2, lhsT=wt[:, (4, 5, 8)[kw], :],
                    rhs=act[:, r + 2:r + 4, kw:kw + W],
                    start=False, stop=(kw == 2),
                )

        if MM_SCHEME == "dr5":
            conv_pair = conv_pair_dr5
        elif MM_SCHEME == "base9":
            conv_pair = conv_pair_base9
        else:                      # 'mix': interleave schemes to sit just
            # under the chip power threshold (dr5 alone trips an ~18%
            # whole-chip downclock; base9 alone wastes PE cycles)
            def conv_pair(ps2, act, r):
                if (r // 2) % MIX_DEN < MIX_NUM:
                    conv_pair_dr5(ps2, act, r)
                else:
                    conv_pair_base9(ps2, act, r)

        SPLIT_FIRST_CHUNK = True
        cur = load_strip(0)
        sign_strip(0, *cur, FIRST_CHUNKS)
        nxt = None
        for s in range(NST):
            h0 = H0S[s]
            HS_S = STRIP_HS[s]
            NG = HS_S // GROUP_ROWS
            xs, act = cur
            ys = ypool.tile([P, HSMAX, W], FP16, name="ys")
            for g in range(NG):
                if g == min(1, NG - 1) and s + 1 < NST:
                    nxt = load_strip(s + 1)   # loads overlap this strip
                if s + 1 < NST:
                    # spread the next strip's sign chunks across this strip's
                    # groups so ACT alternates relu/sign and the next strip's
                    # matmuls never wait at the boundary
                    for ci in range(len(NEXT_CHUNKS)):
                        if g == min(2 * (ci + 1), NG - 1) and g > 1:
                            sign_strip(s + 1, *nxt, NEXT_CHUNKS[ci:ci + 1],
                                       skip=sum(NEXT_CHUNKS[:ci]))
                r = g * GROUP_ROWS
                ps4 = pspool.tile([P, GROUP_ROWS, 256], F32, name="ps")
                conv_pair(ps4[:, 0:2, :], act, r)
                conv_pair(ps4[:, 2:4, :], act, r + 2)
                u4 = ys[:, r:r + GROUP_ROWS, :]
                x4 = xs[:, r + 1:r + 1 + GROUP_ROWS, :]
                if zero_bias:
                    # r = relu((1-slope)*(s*ps + b0)) on ACT;
                    # m = slope*s*ps + r on DVE (single PSUM read);
                    # y = m + x on DVE (all-fp16 SBUF: fast mode)
                    r4 = tpool.tile([P, GROUP_ROWS, W], FP16, name="r")
                    nc.scalar.activation(
                        r4, ps4, AF.Relu,
                        bias=pt[:, PBB:PBB + 1], scale=pt[:, PA:PA + 1],
                    )
                    m4 = tpool.tile([P, GROUP_ROWS, W], FP16, name="m")
                    nc.vector.scalar_tensor_tensor(
                        m4, ps4, pt[:, PD:PD + 1], r4, ALU.mult, ALU.add
                    )
                    nc.vector.tensor_tensor(u4, m4, x4, ALU.add)
                    if with_ec:
                        nc.gpsimd.tensor_scalar(u4, u4, pt[:, PEc:PEc + 1],
                                                None, ALU.add)
                else:
                    # v = ps*s + b0; m = min(v,0)*(slope-1); u = v + m;
                    # y = (u + b1) + x
                    v4 = tpool.tile([P, GROUP_ROWS, W], F32, name="v")
                    nc.vector.tensor_scalar(
                        v4, ps4, pt[:, PS:PS + 1], pt[:, PB0:PB0 + 1],
                        ALU.mult, ALU.add,
                    )
                    m4 = tpool.tile([P, GROUP_ROWS, W], F32, name="mw")
                    nc.vector.tensor_scalar(
                        m4, v4, 0.0, pt[:, PCM:PCM + 1], ALU.min, ALU.mult
                    )
                    nc.vector.tensor_tensor(v4, v4, m4, ALU.add)
                    nc.vector.scalar_tensor_tensor(
                        u4, v4, pt[:, PB1:PB1 + 1], x4, ALU.add, ALU.add
                    )
                # eager stores every 2 groups on the ACT HWDGE ring
                seng = nc.scalar
                if g % 2 == 1:
                    r0s = (g - 1) * GROUP_ROWS
                    seng.dma_start(
                        out=y3[:, h0 + r0s:h0 + r + GROUP_ROWS, :],
                        in_=ys[:, r0s:r + GROUP_ROWS, :])
            if NG % 2 == 1:              # leftover rows of an odd group count
                r0s = (NG - 1) * GROUP_ROWS
                seng.dma_start(out=y3[:, h0 + r0s:h0 + HS_S, :],
                               in_=ys[:, r0s:HS_S, :])
            cur = nxt


def build_nc(zero_bias=True, with_ec=False):
    nc = bacc.Bacc("TRN2", target_bir_lowering=False, debug=False,
                   num_devices=NCORES)
    x_d = nc.dram_tensor("xin", [P, H * W], FP16, kind="ExternalInput").ap()
    w_d = nc.dram_tensor("wp", [P, 9 * 128], FP8, kind="ExternalInput").ap()
    p_d = nc.dram_tensor("pp", [P, NPAR], F32, kind="ExternalInput").ap()
    y_d = nc.dram_tensor("yout", [P, H * W], FP16, kind="ExternalOutput").ap()
    with tile.TileContext(nc) as tc:
        _emit(tc, nc, x_d, w_d, p_d, y_d, zero_bias, with_ec)
    nc.compile()
    return nc


_NC_CACHE = {}


def _get_nc(zero_bias, with_ec=False):
    key = (zero_bias, with_ec)
    if key not in _NC_CACHE:
        _NC_CACHE[key] = build_nc(zero_bias, with_ec)
    return _NC_CACHE[key]


def make_inputs(x, rd_k, rd_b, beta, conv_w, pr_bias0, prelu_w, pr_bias1):
    """Host-side prep: per-channel param table, packed sign weights, shards."""
    k = np.asarray(rd_k, np.float32).reshape(C)
    b = np.asarray(rd_b, np.float32).reshape(C)
    s = np.mean(np.abs(np.asarray(conv_w, np.float32)), axis=(1, 2, 3))
    b0 = np.asarray(pr_bias0, np.float32).reshape(C)
    slope = np.asarray(prelu_w, np.float32).reshape(C)
    b1 = np.asarray(pr_bias1, np.float32).reshape(C)
    cm = slope - 1.0
    cols = np.stack([
        k, b,
        (1.0 - slope) * s, (1.0 - slope) * b0,          # PA, PBB
        slope * s, slope * b0 + b1,                     # PD, PEc
        s, b0, cm, b1,                                  # fallback
        np.zeros(C, np.float32), np.zeros(C, np.float32),
    ], axis=1)
    pp = np.concatenate([cols, cols], axis=0).astype(np.float32)  # [128, 12]

    sw = np.sign(np.asarray(conv_w, np.float32))  # [co, ci, kh, kw]

    def blockdiag(kh, kw):
        S = sw[:, :, kh, kw].T  # [ci, co]
        out = np.zeros((P, P), np.float32)
        out[0:C, 0:C] = S
        out[C:P, C:P] = S
        return out

    wp = np.zeros((P, 9, 128), np.float32)
    for j in range(3):
        for d in range(2):
            wp[:, 2 * j + d, :] = blockdiag(j, d)
    for d in range(2):
        wp[:, 6 + d, :] = blockdiag(d, 2)
    wp[:, 8, :] = blockdiag(2, 2)
    wp = np.ascontiguousarray(wp.reshape(P, 9 * 128)).astype(mybir.dt.np(FP8))

    x = np.asarray(x, np.float32)
    in_maps = []
    for c in range(NCORES):
        xc = np.ascontiguousarray(x[2 * c:2 * c + 2]).reshape(P, H * W)
        in_maps.append({"xin": xc.astype(np.float16), "wp": wp, "pp": pp})
    return in_maps


def kernel(x, rd_k, rd_b, beta, conv_w, pr_bias0, prelu_w, pr_bias1):
    slope = np.asarray(prelu_w, np.float32).reshape(C)
    # relu decomposition of prelu needs (1-slope) >= 0
    zero_bias = bool(np.all((slope >= 0.0) & (slope <= 1.0)))
    ec = (slope * np.asarray(pr_bias0, np.float32).reshape(C)
          + np.asarray(pr_bias1, np.float32).reshape(C))
    with_ec = bool(np.any(ec != 0.0))
    in_maps = make_inputs(x, rd_k, rd_b, beta, conv_w, pr_bias0, prelu_w,
                          pr_bias1)
    nc = _get_nc(zero_bias, with_ec)
    res = run_bass_kernel_spmd(nc, in_maps, core_ids=list(range(NCORES)))
    y = np.empty((B, C, H, W), np.float32)
    for c in range(NCORES):
        y[2 * c:2 * c + 2] = (
            res.results[c]["yout"].astype(np.float32).reshape(2, C, H, W))
    return y


# revision 28
# speedup vs baseline: 1.2199x; 1.0799x over previous
"""Trainium2 Bass kernel for nn_BinaryConv2d (B=16, C=64, H=W=256, 3x3, pad 1).

Forward semantics (STE forward values):
  act = sign(x * rd_k + rd_b)                  in {-1, 0, +1}
  bw  = scaling[co] * sign(conv_w)             scaling = mean |conv_w| per out-ch
  y   = prelu(conv2d(act, bw, pad=1) + pr_bias0) + pr_bias1 + x

Strategy: data-parallel over batch, 2 images per core (8 cores).  The two
images' 64 channels are stacked on the 128 SBUF partitions.  x and y travel
through HBM as fp16 (halves DMA vs fp32; fp16 keeps 11-bit mantissa so the
residual path stays accurate).  Activations are binarized to fp8 +-1 on the
Scalar engine; the 3x3 conv is exact integer arithmetic in fp32 PSUM via
fp8 DoubleRow matmuls.  Per 2-row psum pair: 4 DR matmuls (kw0/kw1 pairs per
kh via a 1-elem delta step, plus the kh0/kh1 pair at kw2) + 1 plain matmul
(kh2,kw2) -- 5 streams instead of the naive 9.  Post-ops: with the graded
zero biases, prelu folds to m = max(ps, slope*ps) (one DVE op from PSUM) and
y = scaling*m + x (one Pool op), keeping ACT free for the sign.
"""

import sys

if "/opt/trn_rl_repo" not in sys.path:
    sys.path.insert(0, "/opt/trn_rl_repo")

from contextlib import ExitStack

import ml_dtypes
import numpy as np

import concourse.bacc as bacc
import concourse.bass as bass
import concourse.tile as tile
from concourse import mybir
from concourse.ap import AP
from concourse.bass_utils import run_bass_kernel_spmd

B, C, H, W = 16, 64, 256, 256
NCORES = 8
P = 128                      # partitions = 2 images x 64 channels

F32 = mybir.dt.float32
FP16 = mybir.dt.float16
FP8 = mybir.dt.float8e4
AF = mybir.ActivationFunctionType
ALU = mybir.AluOpType

APITCH = 272                 # act row pitch (16B-aligned for DR row deltas)

# Param table columns (per-partition f32 scalars).  PA=(1-slope)*s,
# PBB=(1-slope)*b0, PD=slope*s, PE=slope*b0+b1 serve the fused
# prelu(v) = slope*v + relu((1-slope)*v) decomposition (valid slope<=1);
# PS/PB0/PCM/PB1 serve the general fallback.
PK, PB, PA, PBB, PD, PEc, PS, PB0, PCM, PB1 = range(10)
NPAR = 12

STRIP_HS = [32] * 8          # strip heights (sum == H)
GROUP_ROWS = 4               # output rows per PSUM group (2 banks)

# 'dr5': 5 matmuls/pair (4 DR N=512 + 1 plain N=512): fewest PE cycles, but
#   the DR-heavy mix plus full engine activity trips the chip power limit and
#   the whole chip downclocks ~18%.
# 'base9': 6 DR N=256 + 3 plain N=512 per pair: more cycles at full clock.
MM_SCHEME = "mix"
MIX_NUM, MIX_DEN = 7, 8      # fraction of pairs using dr5


def _emit(tc, nc, x_d, w_d, p_d, y_d, zero_bias, with_ec):
    x3 = x_d.rearrange("p (h w) -> p h w", w=W)
    y3 = y_d.rearrange("p (h w) -> p h w", w=W)

    with ExitStack() as ctx:
        consts = ctx.enter_context(tc.tile_pool(name="consts", bufs=1))
        xpool = ctx.enter_context(tc.tile_pool(name="xpool", bufs=2))
        apool = ctx.enter_context(tc.tile_pool(name="apool", bufs=2))
        ypool = ctx.enter_context(tc.tile_pool(name="ypool", bufs=2))
        tpool = ctx.enter_context(tc.tile_pool(name="tpool", bufs=4))
        pspool = ctx.enter_context(tc.tile_pool(name="pspool", bufs=4,
                                                space="PSUM"))

        # params first on the load ring (sign needs them); weights on the
        # store ring, which is idle at kernel start
        pt = consts.tile([P, NPAR], F32)
        nc.sync.dma_start(out=pt, in_=p_d)
        # weight slots: [2j+d] = (kh=j, kw=d in {0,1}); [6+d] = (kh=d, kw=2);
        # [8] = (kh=2, kw=2)
        wt = consts.tile([P, 9, 128], FP8)
        nc.scalar.dma_start(out=wt,
                            in_=w_d.rearrange("p (j m) -> p j m", j=9))

        H0S = [sum(STRIP_HS[:i]) for i in range(len(STRIP_HS))]
        NST = len(STRIP_HS)
        HSMAX = max(STRIP_HS)

        def strip_rows(s):
            h0 = H0S[s]
            row_lo = max(h0 - 1, 0)
            row_hi = min(h0 + STRIP_HS[s] + 1, H)
            return h0, row_lo, row_hi, row_lo - (h0 - 1)

        def load_strip(s):
            """DMA the x strip (rows h0-1 .. h0+hs; tile row a <-> global
            h0-1+a) and memset the act padding."""
            h0, row_lo, row_hi, r0 = strip_rows(s)
            nr = row_hi - row_lo
            xs = xpool.tile([P, HSMAX + 2, W], FP16, name="xs")
            if s == 0:                   # tiny first piece: fast start
                bounds = [0, 3, 9, 18, 27, row_hi]
            else:
                nld = 2
                bounds = [row_lo + (nr * i) // nld for i in range(nld + 1)]
            for a, b in zip(bounds, bounds[1:]):
                if b > a:
                    nc.sync.dma_start(out=xs[:, a - (h0 - 1):b - (h0 - 1), :],
                                      in_=x3[:, a:b, :])
            act = apool.tile([P, HSMAX + 2, APITCH], FP8, name="act")
            nrows = STRIP_HS[s] + 2
            nc.gpsimd.memset(act[:, :nrows, 0:1], 0.0)
            nc.gpsimd.memset(act[:, :nrows, W + 1:W + 2], 0.0)
            if s == 0:
                nc.gpsimd.memset(act[:, 0:1, :], 0.0)
            if s == NST - 1:
                nc.gpsimd.memset(act[:, nrows - 1:nrows, :], 0.0)
            return xs, act

        def sign_strip(s, xs, act, chunks, skip=0):
            """Binarize x into the zero-padded act tile, in row chunks (the
            first small so dependent matmuls unblock quickly)."""
            _, row_lo, row_hi, r0 = strip_rows(s)
            c0 = r0 + skip
            for sz in chunks:
                c1 = min(c0 + sz, r0 + (row_hi - row_lo))
                if c1 <= c0:
                    break
                nc.scalar.activation(
                    act[:, c0:c1, 1:W + 1], xs[:, c0:c1, :], AF.Sign,
                    bias=pt[:, PB:PB + 1], scale=pt[:, PK:PK + 1],
                )
                c0 = c1

        FIRST_CHUNKS = (3, 2, 4, 6, 7, 6, 6)   # strip 0: progressive chunks
        NEXT_CHUNKS = (6, 9, 9, 10)

        ppitch = (HSMAX + 2) * APITCH
        wbase = wt[:, 0, 0]
        woff0 = wbase.offset

        def conv_pair_dr5(ps2, act, r):
            """5 matmuls accumulating the 3x3 binary conv for output rows
            (r, r+1) of the strip into a 2-row psum slice.  act row index
            r+kh+i corresponds to input row (h0-1) + r+kh+i."""
            base = act[:, 0, 0]          # anchor AP for offset math
            off0 = base.offset

            def rhs(row, col, dstep):
                return AP(base.tensor, off0 + row * APITCH + col,
                          [[ppitch, P], [dstep, 2], [APITCH, 2], [1, 256]])

            for j in range(3):           # (kh=j, kw=0)+(kh=j, kw=1) pairs
                nc.tensor.matmul(
                    ps2, lhsT=wt[:, 2 * j:2 * j + 2, :], rhs=rhs(r + j, 0, 1),
                    start=(j == 0), stop=False,
                    perf_mode=mybir.MatmulPerfMode.DoubleRow,
                )
            nc.tensor.matmul(             # (kh=0, kw=2)+(kh=1, kw=2) pair
                ps2, lhsT=wt[:, 6:8, :], rhs=rhs(r, 2, APITCH),
                start=False, stop=False,
                perf_mode=mybir.MatmulPerfMode.DoubleRow,
            )
            nc.tensor.matmul(             # (kh=2, kw=2) plain
                ps2, lhsT=wt[:, 8, :], rhs=act[:, r + 2:r + 4, 2:2 + W],
                start=False, stop=True,
            )

        def conv_pair_base9(ps2, act, r):
            """Baseline scheme: per output row, 3 DR matmuls (kh0+kh1 per kw,
            N=256); then kh2 via 3 plain N=512 matmuls shared by both rows."""
            base = act[:, 0, 0]
            off0 = base.offset

            def wpair(slot_a, stride):
                return AP(wbase.tensor, woff0 + slot_a * 128,
                          [[9 * 128, P], [stride, 2], [1, 128]])

            for i in range(2):           # output row r+i
                po = ps2[:, i, :]
                for kw in range(2):      # (kh0,kh1) DR pairs at kw0/kw1
                    nc.tensor.matmul(
                        po, lhsT=wpair(kw, 256),
                        rhs=AP(base.tensor, off0 + (r + i) * APITCH + kw,
                               [[ppitch, P], [APITCH, 2], [1, 256]]),
                        start=(i == 0 and kw == 0), stop=False,
                        perf_mode=mybir.MatmulPerfMode.DoubleRow,
                    )
                nc.tensor.matmul(        # (kh0,kh1) DR pair at kw2
                    po, lhsT=wt[:, 6:8, :],
                    rhs=AP(base.tensor, off0 + (r + i) * APITCH + 2,
                           [[ppitch, P], [APITCH, 2], [1, 256]]),
                    start=False, stop=False,
                    perf_mode=mybir.MatmulPerfMode.DoubleRow,
                )
            for kw in range(3):          # kh2 plain, both rows
                nc.tensor.matmul(
                    ps2, lhsT=wt[:, (4, 5, 8)[kw], :],
                    rhs=act[:, r + 2:r + 4, kw:kw + W],
                    start=False, stop=(kw == 2),
                )

        if MM_SCHEME == "dr5":
            conv_pair = conv_pair_dr5
        elif MM_SCHEME == "base9":
            conv_pair = conv_pair_base9
        else:                      # 'mix': interleave schemes to sit just
            # under the chip power threshold (dr5 alone trips an ~18%
            # whole-chip downclock; base9 alone wastes PE cycles)
            def conv_pair(ps2, act, r):
                if (r // 2) % MIX_DEN < MIX_NUM:
                    conv_pair_dr5(ps2, act, r)
                else:
                    conv_pair_base9(ps2, act, r)

        SPLIT_FIRST_CHUNK = True
        cur = load_strip(0)
        sign_strip(0, *cur, FIRST_CHUNKS)
        nxt = None
        for s in range(NST):
            h0 = H0S[s]
            HS_S = STRIP_HS[s]
            NG = HS_S // GROUP_ROWS
            xs, act = cur
            ys = ypool.tile([P, HSMAX, W], FP16, name="ys")
            for g in range(NG):
                if g == min(1, NG - 1) and s + 1 < NST:
                    nxt = load_strip(s + 1)   # loads overlap this strip
                if g == max(NG - 2, 0) and s + 1 < NST:
                    # data definitely landed; ACT binarizes it while the PE
                    # finishes this strip
                    sign_strip(s + 1, *nxt, NEXT_CHUNKS[:1])
                r = g * GROUP_ROWS
                ps4 = pspool.tile([P, GROUP_ROWS, 256], F32, name="ps")
                conv_pair(ps4[:, 0:2, :], act, r)
                conv_pair(ps4[:, 2:4, :], act, r + 2)
                u4 = ys[:, r:r + GROUP_ROWS, :]
                x4 = xs[:, r + 1:r + 1 + GROUP_ROWS, :]
                if zero_bias:
                    # r = relu((1-slope)*(s*ps + b0)) on ACT;
                    # m = slope*s*ps + r on DVE (single PSUM read);
                    # y = m + x on DVE (all-fp16 SBUF: fast mode)
                    r4 = tpool.tile([P, GROUP_ROWS, W], FP16, name="r")
                    nc.scalar.activation(
                        r4, ps4, AF.Relu,
                        bias=pt[:, PBB:PBB + 1], scale=pt[:, PA:PA + 1],
                    )
                    m4 = tpool.tile([P, GROUP_ROWS, W], FP16, name="m")
                    nc.vector.scalar_tensor_tensor(
                        m4, ps4, pt[:, PD:PD + 1], r4, ALU.mult, ALU.add
                    )
                    nc.vector.tensor_tensor(u4, m4, x4, ALU.add)
                    if with_ec:
                        nc.gpsimd.tensor_scalar(u4, u4, pt[:, PEc:PEc + 1],
                                                None, ALU.add)
                else:
                    # v = ps*s + b0; m = min(v,0)*(slope-1); u = v + m;
                    # y = (u + b1) + x
                    v4 = tpool.tile([P, GROUP_ROWS, W], F32, name="v")
                    nc.vector.tensor_scalar(
                        v4, ps4, pt[:, PS:PS + 1], pt[:, PB0:PB0 + 1],
                        ALU.mult, ALU.add,
                    )
                    m4 = tpool.tile([P, GROUP_ROWS, W], F32, name="mw")
                    nc.vector.tensor_scalar(
                        m4, v4, 0.0, pt[:, PCM:PCM + 1], ALU.min, ALU.mult
                    )
                    nc.vector.tensor_tensor(v4, v4, m4, ALU.add)
                    nc.vector.scalar_tensor_tensor(
                        u4, v4, pt[:, PB1:PB1 + 1], x4, ALU.add, ALU.add
                    )
                # last strip: eager stores every 2 groups shorten the drain
                if s == NST - 1 and g % 2 == 1:
                    r0s = (g - 1) * GROUP_ROWS
                    nc.scalar.dma_start(
                        out=y3[:, h0 + r0s:h0 + r + GROUP_ROWS, :],
                        in_=ys[:, r0s:r + GROUP_ROWS, :])
            if s == NST - 1:
                if NG % 2 == 1:
                    r0s = (NG - 1) * GROUP_ROWS
                    nc.scalar.dma_start(out=y3[:, h0 + r0s:h0 + HS_S, :],
                                        in_=ys[:, r0s:HS_S, :])
            else:
                for q in range(2):
                    rq = q * (HS_S // 2)
                    rq1 = (q + 1) * (HS_S // 2)
                    nc.scalar.dma_start(out=y3[:, h0 + rq:h0 + rq1, :],
                                        in_=ys[:, rq:rq1, :])
            if s + 1 < NST:
                sign_strip(s + 1, *nxt, NEXT_CHUNKS[1:],
                           skip=NEXT_CHUNKS[0])
            cur = nxt


def build_nc(zero_bias=True, with_ec=False):
    nc = bacc.Bacc("TRN2", target_bir_lowering=False, debug=False,
                   num_devices=NCORES)
    x_d = nc.dram_tensor("xin", [P, H * W], FP16, kind="ExternalInput").ap()
    w_d = nc.dram_tensor("wp", [P, 9 * 128], FP8, kind="ExternalInput").ap()
    p_d = nc.dram_tensor("pp", [P, NPAR], F32, kind="ExternalInput").ap()
    y_d = nc.dram_tensor("yout", [P, H * W], FP16, kind="ExternalOutput").ap()
    with tile.TileContext(nc) as tc:
        _emit(tc, nc, x_d, w_d, p_d, y_d, zero_bias, with_ec)
    nc.compile()
    return nc


_NC_CACHE = {}


def _get_nc(zero_bias, with_ec=False):
    key = (zero_bias, with_ec)
    if key not in _NC_CACHE:
        _NC_CACHE[key] = build_nc(zero_bias, with_ec)
    return _NC_CACHE[key]


def make_inputs(x, rd_k, rd_b, beta, conv_w, pr_bias0, prelu_w, pr_bias1):
    """Host-side prep: per-channel param table, packed sign weights, shards."""
    k = np.asarray(rd_k, np.float32).reshape(C)
    b = np.asarray(rd_b, np.float32).reshape(C)
    s = np.mean(np.abs(np.asarray(conv_w, np.float32)), axis=(1, 2, 3))
    b0 = np.asarray(pr_bias0, np.float32).reshape(C)
    slope = np.asarray(prelu_w, np.float32).reshape(C)
    b1 = np.asarray(pr_bias1, np.float32).reshape(C)
    cm = slope - 1.0
    cols = np.stack([
        k, b,
        (1.0 - slope) * s, (1.0 - slope) * b0,          # PA, PBB
        slope * s, slope * b0 + b1,                     # PD, PEc
        s, b0, cm, b1,                                  # fallback
        np.zeros(C, np.float32), np.zeros(C, np.float32),
    ], axis=1)
    pp = np.concatenate([cols, cols], axis=0).astype(np.float32)  # [128, 12]

    sw = np.sign(np.asarray(conv_w, np.float32))  # [co, ci, kh, kw]

    def blockdiag(kh, kw):
        S = sw[:, :, kh, kw].T  # [ci, co]
        out = np.zeros((P, P), np.float32)
        out[0:C, 0:C] = S
        out[C:P, C:P] = S
        return out

    wp = np.zeros((P, 9, 128), np.float32)
    for j in range(3):
        for d in range(2):
            wp[:, 2 * j + d, :] = blockdiag(j, d)
    for d in range(2):
        wp[:, 6 + d, :] = blockdiag(d, 2)
    wp[:, 8, :] = blockdiag(2, 2)
    wp = np.ascontiguousarray(wp.reshape(P, 9 * 128)).astype(mybir.dt.np(FP8))

    x = np.asarray(x, np.float32)
    in_maps = []
    for c in range(NCORES):
        xc = np.ascontiguousarray(x[2 * c:2 * c + 2]).reshape(P, H * W)
        in_maps.append({"xin": xc.astype(np.float16), "wp": wp, "pp": pp})
    return in_maps


def kernel(x, rd_k, rd_b, beta, conv_w, pr_bias0, prelu_w, pr_bias1):
    slope = np.asarray(prelu_w, np.float32).reshape(C)
    # relu decomposition of prelu needs (1-slope) >= 0
    zero_bias = bool(np.all((slope >= 0.0) & (slope <= 1.0)))
    ec = (slope * np.asarray(pr_bias0, np.float32).reshape(C)
          + np.asarray(pr_bias1, np.float32).reshape(C))
    with_ec = bool(np.any(ec != 0.0))
    in_maps = make_inputs(x, rd_k, rd_b, beta, conv_w, pr_bias0, prelu_w,
                          pr_bias1)
    nc = _get_nc(zero_bias, with_ec)
    res = run_bass_kernel_spmd(nc, in_maps, core_ids=list(range(NCORES)))
    y = np.empty((B, C, H, W), np.float32)
    for c in range(NCORES):
        y[2 * c:2 * c + 2] = (
            res.results[c]["yout"].astype(np.float32).reshape(2, C, H, W))
    return y


# revision 29
# speedup vs baseline: 1.2303x; 1.0085x over previous
"""Trainium2 Bass kernel for nn_BinaryConv2d (B=16, C=64, H=W=256, 3x3, pad 1).

Forward semantics (STE forward values):
  act = sign(x * rd_k + rd_b)                  in {-1, 0, +1}
  bw  = scaling[co] * sign(conv_w)             scaling = mean |conv_w| per out-ch
  y   = prelu(conv2d(act, bw, pad=1) + pr_bias0) + pr_bias1 + x

Strategy: data-parallel over batch, 2 images per core (8 cores).  The two
images' 64 channels are stacked on the 128 SBUF partitions.  x and y travel
through HBM as fp16 (halves DMA vs fp32; fp16 keeps 11-bit mantissa so the
residual path stays accurate).  Activations are binarized to fp8 +-1 on the
Scalar engine; the 3x3 conv is exact integer arithmetic in fp32 PSUM via
fp8 DoubleRow matmuls.  Per 2-row psum pair: 4 DR matmuls (kw0/kw1 pairs per
kh via a 1-elem delta step, plus the kh0/kh1 pair at kw2) + 1 plain matmul
(kh2,kw2) -- 5 streams instead of the naive 9.  Post-ops: with the graded
zero biases, prelu folds to m = max(ps, slope*ps) (one DVE op from PSUM) and
y = scaling*m + x (one Pool op), keeping ACT free for the sign.
"""

import sys

if "/opt/trn_rl_repo" not in sys.path:
    sys.path.insert(0, "/opt/trn_rl_repo")

from contextlib import ExitStack

import ml_dtypes
import numpy as np

import concourse.bacc as bacc
import concourse.bass as bass
import concourse.tile as tile
from concourse import mybir
from concourse.ap import AP
from concourse.bass_utils import run_bass_kernel_spmd

B, C, H, W = 16, 64, 256, 256
NCORES = 8
P = 128                      # partitions = 2 images x 64 channels

F32 = mybir.dt.float32
FP16 = mybir.dt.float16
FP8 = mybir.dt.float8e4
AF = mybir.ActivationFunctionType
ALU = mybir.AluOpType

APITCH = 272                 # act row pitch (16B-aligned for DR row deltas)

# Param table columns (per-partition f32 scalars).  PA=(1-slope)*s,
# PBB=(1-slope)*b0, PD=slope*s, PE=slope*b0+b1 serve the fused
# prelu(v) = slope*v + relu((1-slope)*v) decomposition (valid slope<=1);
# PS/PB0/PCM/PB1 serve the general fallback.
PK, PB, PA, PBB, PD, PEc, PS, PB0, PCM, PB1 = range(10)
NPAR = 12

STRIP_HS = [32] * 8          # strip heights (sum == H)
GROUP_ROWS = 4               # output rows per PSUM group (2 banks)

# 'dr5': 5 matmuls/pair (4 DR N=512 + 1 plain N=512): fewest PE cycles, but
#   the DR-heavy mix plus full engine activity trips the chip power limit and
#   the whole chip downclocks ~18%.
# 'base9': 6 DR N=256 + 3 plain N=512 per pair: more cycles at full clock.
MM_SCHEME = "mix"
MIX_NUM, MIX_DEN = 7, 8      # fraction of pairs using dr5


def _emit(tc, nc, x_d, w_d, p_d, y_d, zero_bias, with_ec):
    x3 = x_d.rearrange("p (h w) -> p h w", w=W)
    y3 = y_d.rearrange("p (h w) -> p h w", w=W)

    with ExitStack() as ctx:
        consts = ctx.enter_context(tc.tile_pool(name="consts", bufs=1))
        xpool = ctx.enter_context(tc.tile_pool(name="xpool", bufs=2))
        apool = ctx.enter_context(tc.tile_pool(name="apool", bufs=2))
        ypool = ctx.enter_context(tc.tile_pool(name="ypool", bufs=2))
        tpool = ctx.enter_context(tc.tile_pool(name="tpool", bufs=4))
        pspool = ctx.enter_context(tc.tile_pool(name="pspool", bufs=4,
                                                space="PSUM"))

        # params first on the load ring (sign needs them); weights on the
        # store ring, which is idle at kernel start
        pt = consts.tile([P, NPAR], F32)
        nc.sync.dma_start(out=pt, in_=p_d)
        # weight slots: [2j+d] = (kh=j, kw=d in {0,1}); [6+d] = (kh=d, kw=2);
        # [8] = (kh=2, kw=2)
        wt = consts.tile([P, 9, 128], FP8)
        nc.scalar.dma_start(out=wt,
                            in_=w_d.rearrange("p (j m) -> p j m", j=9))

        H0S = [sum(STRIP_HS[:i]) for i in range(len(STRIP_HS))]
        NST = len(STRIP_HS)
        HSMAX = max(STRIP_HS)

        def strip_rows(s):
            h0 = H0S[s]
            row_lo = max(h0 - 1, 0)
            row_hi = min(h0 + STRIP_HS[s] + 1, H)
            return h0, row_lo, row_hi, row_lo - (h0 - 1)

        def load_strip(s):
            """DMA the x strip (rows h0-1 .. h0+hs; tile row a <-> global
            h0-1+a) and memset the act padding."""
            h0, row_lo, row_hi, r0 = strip_rows(s)
            nr = row_hi - row_lo
            xs = xpool.tile([P, HSMAX + 2, W], FP16, name="xs")
            if s == 0:                   # tiny first piece: fast start
                bounds = [0, 3, 9, 18, 27, row_hi]
            else:
                nld = 2
                bounds = [row_lo + (nr * i) // nld for i in range(nld + 1)]
            for a, b in zip(bounds, bounds[1:]):
                if b > a:
                    nc.sync.dma_start(out=xs[:, a - (h0 - 1):b - (h0 - 1), :],
                                      in_=x3[:, a:b, :])
            act = apool.tile([P, HSMAX + 2, APITCH], FP8, name="act")
            nrows = STRIP_HS[s] + 2
            nc.gpsimd.memset(act[:, :nrows, 0:1], 0.0)
            nc.gpsimd.memset(act[:, :nrows, W + 1:W + 2], 0.0)
            if s == 0:
                nc.gpsimd.memset(act[:, 0:1, :], 0.0)
            if s == NST - 1:
                nc.gpsimd.memset(act[:, nrows - 1:nrows, :], 0.0)
            return xs, act

        def sign_strip(s, xs, act, chunks, skip=0):
            """Binarize x into the zero-padded act tile, in row chunks (the
            first small so dependent matmuls unblock quickly)."""
            _, row_lo, row_hi, r0 = strip_rows(s)
            c0 = r0 + skip
            for sz in chunks:
                c1 = min(c0 + sz, r0 + (row_hi - row_lo))
                if c1 <= c0:
                    break
                nc.scalar.activation(
                    act[:, c0:c1, 1:W + 1], xs[:, c0:c1, :], AF.Sign,
                    bias=pt[:, PB:PB + 1], scale=pt[:, PK:PK + 1],
                )
                c0 = c1

        FIRST_CHUNKS = (3, 2, 4, 6, 7, 6, 6)   # strip 0: progressive chunks
        NEXT_CHUNKS = (6, 9, 9, 10)

        ppitch = (HSMAX + 2) * APITCH
        wbase = wt[:, 0, 0]
        woff0 = wbase.offset

        def conv_pair_dr5(ps2, act, r):
            """5 matmuls accumulating the 3x3 binary conv for output rows
            (r, r+1) of the strip into a 2-row psum slice.  act row index
            r+kh+i corresponds to input row (h0-1) + r+kh+i."""
            base = act[:, 0, 0]          # anchor AP for offset math
            off0 = base.offset

            def rhs(row, col, dstep):
                return AP(base.tensor, off0 + row * APITCH + col,
                          [[ppitch, P], [dstep, 2], [APITCH, 2], [1, 256]])

            for j in range(3):           # (kh=j, kw=0)+(kh=j, kw=1) pairs
                nc.tensor.matmul(
                    ps2, lhsT=wt[:, 2 * j:2 * j + 2, :], rhs=rhs(r + j, 0, 1),
                    start=(j == 0), stop=False,
                    perf_mode=mybir.MatmulPerfMode.DoubleRow,
                )
            nc.tensor.matmul(             # (kh=0, kw=2)+(kh=1, kw=2) pair
                ps2, lhsT=wt[:, 6:8, :], rhs=rhs(r, 2, APITCH),
                start=False, stop=False,
                perf_mode=mybir.MatmulPerfMode.DoubleRow,
            )
            nc.tensor.matmul(             # (kh=2, kw=2) plain
                ps2, lhsT=wt[:, 8, :], rhs=act[:, r + 2:r + 4, 2:2 + W],
                start=False, stop=True,
            )

        def conv_pair_base9(ps2, act, r):
            """Baseline scheme: per output row, 3 DR matmuls (kh0+kh1 per kw,
            N=256); then kh2 via 3 plain N=512 matmuls shared by both rows."""
            base = act[:, 0, 0]
            off0 = base.offset

            def wpair(slot_a, stride):
                return AP(wbase.tensor, woff0 + slot_a * 128,
                          [[9 * 128, P], [stride, 2], [1, 128]])

            for i in range(2):           # output row r+i
                po = ps2[:, i, :]
                for kw in range(2):      # (kh0,kh1) DR pairs at kw0/kw1
                    nc.tensor.matmul(
                        po, lhsT=wpair(kw, 256),
                        rhs=AP(base.tensor, off0 + (r + i) * APITCH + kw,
                               [[ppitch, P], [APITCH, 2], [1, 256]]),
                        start=(i == 0 and kw == 0), stop=False,
                        perf_mode=mybir.MatmulPerfMode.DoubleRow,
                    )
                nc.tensor.matmul(        # (kh0,kh1) DR pair at kw2
                    po, lhsT=wt[:, 6:8, :],
                    rhs=AP(base.tensor, off0 + (r + i) * APITCH + 2,
                           [[ppitch, P], [APITCH, 2], [1, 256]]),
                    start=False, stop=False,
                    perf_mode=mybir.MatmulPerfMode.DoubleRow,
                )
            for kw in range(3):          # kh2 plain, both rows
                nc.tensor.matmul(
                    ps2, lhsT=wt[:, (4, 5, 8)[kw], :],
                    rhs=act[:, r + 2:r + 4, kw:kw + W],
                    start=False, stop=(kw == 2),
                )

        if MM_SCHEME == "dr5":
            conv_pair = conv_pair_dr5
        elif MM_SCHEME == "base9":
            conv_pair = conv_pair_base9
        else:                      # 'mix': interleave schemes to sit just
            # under the chip power threshold (dr5 alone trips an ~18%
            # whole-chip downclock; base9 alone wastes PE cycles)
            def conv_pair(ps2, act, r):
                if (r // 2) % MIX_DEN < MIX_NUM:
                    conv_pair_dr5(ps2, act, r)
                else:
                    conv_pair_base9(ps2, act, r)

        SPLIT_FIRST_CHUNK = True
        cur = load_strip(0)
        sign_strip(0, *cur, FIRST_CHUNKS)
        nxt = None
        for s in range(NST):
            h0 = H0S[s]
            HS_S = STRIP_HS[s]
            NG = HS_S // GROUP_ROWS
            xs, act = cur
            ys = ypool.tile([P, HSMAX, W], FP16, name="ys")
            for g in range(NG):
                if g == min(1, NG - 1) and s + 1 < NST:
                    nxt = load_strip(s + 1)   # loads overlap this strip
                if g == max(NG - 4, 0) and s + 1 < NST:
                    # data definitely landed; ACT binarizes it while the PE
                    # finishes this strip
                    sign_strip(s + 1, *nxt, NEXT_CHUNKS[:1])
                if g == max(NG - 2, 0) and s + 1 < NST:
                    sign_strip(s + 1, *nxt, NEXT_CHUNKS[1:2],
                               skip=NEXT_CHUNKS[0])
                r = g * GROUP_ROWS
                ps4 = pspool.tile([P, GROUP_ROWS, 256], F32, name="ps")
                conv_pair(ps4[:, 0:2, :], act, r)
                conv_pair(ps4[:, 2:4, :], act, r + 2)
                u4 = ys[:, r:r + GROUP_ROWS, :]
                x4 = xs[:, r + 1:r + 1 + GROUP_ROWS, :]
                if zero_bias:
                    # r = relu((1-slope)*(s*ps + b0)) on ACT;
                    # m = slope*s*ps + r on DVE (single PSUM read);
                    # y = m + x on DVE (all-fp16 SBUF: fast mode)
                    r4 = tpool.tile([P, GROUP_ROWS, W], FP16, name="r")
                    nc.scalar.activation(
                        r4, ps4, AF.Relu,
                        bias=pt[:, PBB:PBB + 1], scale=pt[:, PA:PA + 1],
                    )
                    m4 = tpool.tile([P, GROUP_ROWS, W], FP16, name="m")
                    nc.vector.scalar_tensor_tensor(
                        m4, ps4, pt[:, PD:PD + 1], r4, ALU.mult, ALU.add
                    )
                    nc.vector.tensor_tensor(u4, m4, x4, ALU.add)
                    if with_ec:
                        nc.gpsimd.tensor_scalar(u4, u4, pt[:, PEc:PEc + 1],
                                                None, ALU.add)
                else:
                    # v = ps*s + b0; m = min(v,0)*(slope-1); u = v + m;
                    # y = (u + b1) + x
                    v4 = tpool.tile([P, GROUP_ROWS, W], F32, name="v")
                    nc.vector.tensor_scalar(
                        v4, ps4, pt[:, PS:PS + 1], pt[:, PB0:PB0 + 1],
                        ALU.mult, ALU.add,
                    )
                    m4 = tpool.tile([P, GROUP_ROWS, W], F32, name="mw")
                    nc.vector.tensor_scalar(
                        m4, v4, 0.0, pt[:, PCM:PCM + 1], ALU.min, ALU.mult
                    )
                    nc.vector.tensor_tensor(v4, v4, m4, ALU.add)
                    nc.vector.scalar_tensor_tensor(
                        u4, v4, pt[:, PB1:PB1 + 1], x4, ALU.add, ALU.add
                    )
                # last strip: eager stores every 2 groups shorten the drain
                if s == NST - 1 and g % 2 == 1:
                    r0s = (g - 1) * GROUP_ROWS
                    nc.scalar.dma_start(
                        out=y3[:, h0 + r0s:h0 + r + GROUP_ROWS, :],
                        in_=ys[:, r0s:r + GROUP_ROWS, :])
            if s == NST - 1:
                if NG % 2 == 1:
                    r0s = (NG - 1) * GROUP_ROWS
                    nc.scalar.dma_start(out=y3[:, h0 + r0s:h0 + HS_S, :],
                                        in_=ys[:, r0s:HS_S, :])
            else:
                for q in range(2):
                    rq = q * (HS_S // 2)
                    rq1 = (q + 1) * (HS_S // 2)
                    nc.scalar.dma_start(out=y3[:, h0 + rq:h0 + rq1, :],
                                        in_=ys[:, rq:rq1, :])
            if s + 1 < NST:
                sign_strip(s + 1, *nxt, NEXT_CHUNKS[2:],
                           skip=sum(NEXT_CHUNKS[:2]))
            cur = nxt


def build_nc(zero_bias=True, with_ec=False):
    nc = bacc.Bacc("TRN2", target_bir_lowering=False, debug=False,
                   num_devices=NCORES)
    x_d = nc.dram_tensor("xin", [P, H * W], FP16, kind="ExternalInput").ap()
    w_d = nc.dram_tensor("wp", [P, 9 * 128], FP8, kind="ExternalInput").ap()
    p_d = nc.dram_tensor("pp", [P, NPAR], F32, kind="ExternalInput").ap()
    y_d = nc.dram_tensor("yout", [P, H * W], FP16, kind="ExternalOutput").ap()
    with tile.TileContext(nc) as tc:
        _emit(tc, nc, x_d, w_d, p_d, y_d, zero_bias, with_ec)
    nc.compile()
    return nc


_NC_CACHE = {}


def _get_nc(zero_bias, with_ec=False):
    key = (zero_bias, with_ec)
    if key not in _NC_CACHE:
        _NC_CACHE[key] = build_nc(zero_bias, with_ec)
    return _NC_CACHE[key]


def make_inputs(x, rd_k, rd_b, beta, conv_w, pr_bias0, prelu_w, pr_bias1):
    """Host-side prep: per-channel param table, packed sign weights, shards."""
    k = np.asarray(rd_k, np.float32).reshape(C)
    b = np.asarray(rd_b, np.float32).reshape(C)
    s = np.mean(np.abs(np.asarray(conv_w, np.float32)), axis=(1, 2, 3))
    b0 = np.asarray(pr_bias0, np.float32).reshape(C)
    slope = np.asarray(prelu_w, np.float32).reshape(C)
    b1 = np.asarray(pr_bias1, np.float32).reshape(C)
    cm = slope - 1.0
    cols = np.stack([
        k, b,
        (1.0 - slope) * s, (1.0 - slope) * b0,          # PA, PBB
        slope * s, slope * b0 + b1,                     # PD, PEc
        s, b0, cm, b1,                                  # fallback
        np.zeros(C, np.float32), np.zeros(C, np.float32),
    ], axis=1)
    pp = np.concatenate([cols, cols], axis=0).astype(np.float32)  # [128, 12]

    sw = np.sign(np.asarray(conv_w, np.float32))  # [co, ci, kh, kw]

    def blockdiag(kh, kw):
        S = sw[:, :, kh, kw].T  # [ci, co]
        out = np.zeros((P, P), np.float32)
        out[0:C, 0:C] = S
        out[C:P, C:P] = S
        return out

    wp = np.zeros((P, 9, 128), np.float32)
    for j in range(3):
        for d in range(2):
            wp[:, 2 * j + d, :] = blockdiag(j, d)
    for d in range(2):
        wp[:, 6 + d, :] = blockdiag(d, 2)
    wp[:, 8, :] = blockdiag(2, 2)
    wp = np.ascontiguousarray(wp.reshape(P, 9 * 128)).astype(mybir.dt.np(FP8))

    x = np.asarray(x, np.float32)
    in_maps = []
    for c in range(NCORES):
        xc = np.ascontiguousarray(x[2 * c:2 * c + 2]).reshape(P, H * W)
        in_maps.append({"xin": xc.astype(np.float16), "wp": wp, "pp": pp})
    return in_maps


def kernel(x, rd_k, rd_b, beta, conv_w, pr_bias0, prelu_w, pr_bias1):
    slope = np.asarray(prelu_w, np.float32).reshape(C)
    # relu decomposition of prelu needs (1-slope) >= 0
    zero_bias = bool(np.all((slope >= 0.0) & (slope <= 1.0)))
    ec = (slope * np.asarray(pr_bias0, np.float32).reshape(C)
          + np.asarray(pr_bias1, np.float32).reshape(C))
    with_ec = bool(np.any(ec != 0.0))
    in_maps = make_inputs(x, rd_k, rd_b, beta, conv_w, pr_bias0, prelu_w,
                          pr_bias1)
    nc = _get_nc(zero_bias, with_ec)
    res = run_bass_kernel_spmd(nc, in_maps, core_ids=list(range(NCORES)))
    y = np.empty((B, C, H, W), np.float32)
    for c in range(NCORES):
        y[2 * c:2 * c + 2] = (
            res.results[c]["yout"].astype(np.float32).reshape(2, C, H, W))
    return y


# revision 30
# speedup vs baseline: 1.2385x; 1.0067x over previous
"""Trainium2 Bass kernel for nn_BinaryConv2d (B=16, C=64, H=W=256, 3x3, pad 1).

Forward semantics (STE forward values):
  act = sign(x * rd_k + rd_b)                  in {-1, 0, +1}
  bw  = scaling[co] * sign(conv_w)             scaling = mean |conv_w| per out-ch
  y   = prelu(conv2d(act, bw, pad=1) + pr_bias0) + pr_bias1 + x

Strategy: data-parallel over batch, 2 images per core (8 cores).  The two
images' 64 channels are stacked on the 128 SBUF partitions.  x and y travel
through HBM as fp16 (halves DMA vs fp32; fp16 keeps 11-bit mantissa so the
residual path stays accurate).  Activations are binarized to fp8 +-1 on the
Scalar engine; the 3x3 conv is exact integer arithmetic in fp32 PSUM via
fp8 DoubleRow matmuls.  Per 2-row psum pair: 4 DR matmuls (kw0/kw1 pairs per
kh via a 1-elem delta step, plus the kh0/kh1 pair at kw2) + 1 plain matmul
(kh2,kw2) -- 5 streams instead of the naive 9.  Post-ops: with the graded
zero biases, prelu folds to m = max(ps, slope*ps) (one DVE op from PSUM) and
y = scaling*m + x (one Pool op), keeping ACT free for the sign.
"""

import sys

if "/opt/trn_rl_repo" not in sys.path:
    sys.path.insert(0, "/opt/trn_rl_repo")

from contextlib import ExitStack

import ml_dtypes
import numpy as np

import concourse.bacc as bacc
import concourse.bass as bass
import concourse.tile as tile
from concourse import mybir
from concourse.ap import AP
from concourse.bass_utils import run_bass_kernel_spmd

B, C, H, W = 16, 64, 256, 256
NCORES = 8
P = 128                      # partitions = 2 images x 64 channels

F32 = mybir.dt.float32
FP16 = mybir.dt.float16
FP8 = mybir.dt.float8e4
AF = mybir.ActivationFunctionType
ALU = mybir.AluOpType

APITCH = 272                 # act row pitch (16B-aligned for DR row deltas)

# Param table columns (per-partition f32 scalars).  PA=(1-slope)*s,
# PBB=(1-slope)*b0, PD=slope*s, PE=slope*b0+b1 serve the fused
# prelu(v) = slope*v + relu((1-slope)*v) decomposition (valid slope<=1);
# PS/PB0/PCM/PB1 serve the general fallback.
PK, PB, PA, PBB, PD, PEc, PS, PB0, PCM, PB1 = range(10)
NPAR = 12

STRIP_HS = [32] * 8          # strip heights (sum == H)
GROUP_ROWS = 4               # output rows per PSUM group (2 banks)

# 'dr5': 5 matmuls/pair (4 DR N=512 + 1 plain N=512): fewest PE cycles, but
#   the DR-heavy mix plus full engine activity trips the chip power limit and
#   the whole chip downclocks ~18%.
# 'base9': 6 DR N=256 + 3 plain N=512 per pair: more cycles at full clock.
MM_SCHEME = "mix"
MIX_NUM, MIX_DEN = 15, 16      # fraction of pairs using dr5


def _emit(tc, nc, x_d, w_d, p_d, y_d, zero_bias, with_ec):
    x3 = x_d.rearrange("p (h w) -> p h w", w=W)
    y3 = y_d.rearrange("p (h w) -> p h w", w=W)

    with ExitStack() as ctx:
        consts = ctx.enter_context(tc.tile_pool(name="consts", bufs=1))
        xpool = ctx.enter_context(tc.tile_pool(name="xpool", bufs=2))
        apool = ctx.enter_context(tc.tile_pool(name="apool", bufs=2))
        ypool = ctx.enter_context(tc.tile_pool(name="ypool", bufs=2))
        tpool = ctx.enter_context(tc.tile_pool(name="tpool", bufs=4))
        pspool = ctx.enter_context(tc.tile_pool(name="pspool", bufs=4,
                                                space="PSUM"))

        # params first on the load ring (sign needs them); weights on the
        # store ring, which is idle at kernel start
        pt = consts.tile([P, NPAR], F32)
        nc.sync.dma_start(out=pt, in_=p_d)
        # weight slots: [2j+d] = (kh=j, kw=d in {0,1}); [6+d] = (kh=d, kw=2);
        # [8] = (kh=2, kw=2)
        wt = consts.tile([P, 9, 128], FP8)
        nc.scalar.dma_start(out=wt,
                            in_=w_d.rearrange("p (j m) -> p j m", j=9))

        H0S = [sum(STRIP_HS[:i]) for i in range(len(STRIP_HS))]
        NST = len(STRIP_HS)
        HSMAX = max(STRIP_HS)

        def strip_rows(s):
            h0 = H0S[s]
            row_lo = max(h0 - 1, 0)
            row_hi = min(h0 + STRIP_HS[s] + 1, H)
            return h0, row_lo, row_hi, row_lo - (h0 - 1)

        def load_strip(s):
            """DMA the x strip (rows h0-1 .. h0+hs; tile row a <-> global
            h0-1+a) and memset the act padding."""
            h0, row_lo, row_hi, r0 = strip_rows(s)
            nr = row_hi - row_lo
            xs = xpool.tile([P, HSMAX + 2, W], FP16, name="xs")
            if s == 0:                   # tiny first piece: fast start
                bounds = [0, 3, 9, 18, 27, row_hi]
            else:
                nld = 2
                bounds = [row_lo + (nr * i) // nld for i in range(nld + 1)]
            for a, b in zip(bounds, bounds[1:]):
                if b > a:
                    nc.sync.dma_start(out=xs[:, a - (h0 - 1):b - (h0 - 1), :],
                                      in_=x3[:, a:b, :])
            act = apool.tile([P, HSMAX + 2, APITCH], FP8, name="act")
            nrows = STRIP_HS[s] + 2
            nc.gpsimd.memset(act[:, :nrows, 0:1], 0.0)
            nc.gpsimd.memset(act[:, :nrows, W + 1:W + 2], 0.0)
            if s == 0:
                nc.gpsimd.memset(act[:, 0:1, :], 0.0)
            if s == NST - 1:
                nc.gpsimd.memset(act[:, nrows - 1:nrows, :], 0.0)
            return xs, act

        def sign_strip(s, xs, act, chunks, skip=0):
            """Binarize x into the zero-padded act tile, in row chunks (the
            first small so dependent matmuls unblock quickly)."""
            _, row_lo, row_hi, r0 = strip_rows(s)
            c0 = r0 + skip
            for sz in chunks:
                c1 = min(c0 + sz, r0 + (row_hi - row_lo))
                if c1 <= c0:
                    break
                nc.scalar.activation(
                    act[:, c0:c1, 1:W + 1], xs[:, c0:c1, :], AF.Sign,
                    bias=pt[:, PB:PB + 1], scale=pt[:, PK:PK + 1],
                )
                c0 = c1

        FIRST_CHUNKS = (3, 2, 4, 6, 7, 6, 6)   # strip 0: progressive chunks
        NEXT_CHUNKS = (6, 9, 9, 10)

        ppitch = (HSMAX + 2) * APITCH
        wbase = wt[:, 0, 0]
        woff0 = wbase.offset

        def conv_pair_dr5(ps2, act, r):
            """5 matmuls accumulating the 3x3 binary conv for output rows
            (r, r+1) of the strip into a 2-row psum slice.  act row index
            r+kh+i corresponds to input row (h0-1) + r+kh+i."""
            base = act[:, 0, 0]          # anchor AP for offset math
            off0 = base.offset

            def rhs(row, col, dstep):
                return AP(base.tensor, off0 + row * APITCH + col,
                          [[ppitch, P], [dstep, 2], [APITCH, 2], [1, 256]])

            for j in range(3):           # (kh=j, kw=0)+(kh=j, kw=1) pairs
                nc.tensor.matmul(
                    ps2, lhsT=wt[:, 2 * j:2 * j + 2, :], rhs=rhs(r + j, 0, 1),
                    start=(j == 0), stop=False,
                    perf_mode=mybir.MatmulPerfMode.DoubleRow,
                )
            nc.tensor.matmul(             # (kh=0, kw=2)+(kh=1, kw=2) pair
                ps2, lhsT=wt[:, 6:8, :], rhs=rhs(r, 2, APITCH),
                start=False, stop=False,
                perf_mode=mybir.MatmulPerfMode.DoubleRow,
            )
            nc.tensor.matmul(             # (kh=2, kw=2) plain
                ps2, lhsT=wt[:, 8, :], rhs=act[:, r + 2:r + 4, 2:2 + W],
                start=False, stop=True,
            )

        def conv_pair_base9(ps2, act, r):
            """Baseline scheme: per output row, 3 DR matmuls (kh0+kh1 per kw,
            N=256); then kh2 via 3 plain N=512 matmuls shared by both rows."""
            base = act[:, 0, 0]
            off0 = base.offset

            def wpair(slot_a, stride):
                return AP(wbase.tensor, woff0 + slot_a * 128,
                          [[9 * 128, P], [stride, 2], [1, 128]])

            for i in range(2):           # output row r+i
                po = ps2[:, i, :]
                for kw in range(2):      # (kh0,kh1) DR pairs at kw0/kw1
                    nc.tensor.matmul(
                        po, lhsT=wpair(kw, 256),
                        rhs=AP(base.tensor, off0 + (r + i) * APITCH + kw,
                               [[ppitch, P], [APITCH, 2], [1, 256]]),
                        start=(i == 0 and kw == 0), stop=False,
                        perf_mode=mybir.MatmulPerfMode.DoubleRow,
                    )
                nc.tensor.matmul(        # (kh0,kh1) DR pair at kw2
                    po, lhsT=wt[:, 6:8, :],
                    rhs=AP(base.tensor, off0 + (r + i) * APITCH + 2,
                           [[ppitch, P], [APITCH, 2], [1, 256]]),
                    start=False, stop=False,
                    perf_mode=mybir.MatmulPerfMode.DoubleRow,
                )
            for kw in range(3):          # kh2 plain, both rows
                nc.tensor.matmul(
                    ps2, lhsT=wt[:, (4, 5, 8)[kw], :],
                    rhs=act[:, r + 2:r + 4, kw:kw + W],
                    start=False, stop=(kw == 2),
                )

        if MM_SCHEME == "dr5":
            conv_pair = conv_pair_dr5
        elif MM_SCHEME == "base9":
            conv_pair = conv_pair_base9
        else:                      # 'mix': interleave schemes to sit just
            # under the chip power threshold (dr5 alone trips an ~18%
            # whole-chip downclock; base9 alone wastes PE cycles)
            def conv_pair(ps2, act, r):
                if (r // 2) % MIX_DEN < MIX_NUM:
                    conv_pair_dr5(ps2, act, r)
                else:
                    conv_pair_base9(ps2, act, r)

        SPLIT_FIRST_CHUNK = True
        cur = load_strip(0)
        sign_strip(0, *cur, FIRST_CHUNKS)
        nxt = None
        for s in range(NST):
            h0 = H0S[s]
            HS_S = STRIP_HS[s]
            NG = HS_S // GROUP_ROWS
            xs, act = cur
            ys = ypool.tile([P, HSMAX, W], FP16, name="ys")
            for g in range(NG):
                if g == min(1, NG - 1) and s + 1 < NST:
                    nxt = load_strip(s + 1)   # loads overlap this strip
                if g == max(NG - 4, 0) and s + 1 < NST:
                    # data definitely landed; ACT binarizes it while the PE
                    # finishes this strip
                    sign_strip(s + 1, *nxt, NEXT_CHUNKS[:1])
                if g == max(NG - 2, 0) and s + 1 < NST:
                    sign_strip(s + 1, *nxt, NEXT_CHUNKS[1:2],
                               skip=NEXT_CHUNKS[0])
                r = g * GROUP_ROWS
                ps4 = pspool.tile([P, GROUP_ROWS, 256], F32, name="ps")
                conv_pair(ps4[:, 0:2, :], act, r)
                conv_pair(ps4[:, 2:4, :], act, r + 2)
                u4 = ys[:, r:r + GROUP_ROWS, :]
                x4 = xs[:, r + 1:r + 1 + GROUP_ROWS, :]
                if zero_bias:
                    # r = relu((1-slope)*(s*ps + b0)) on ACT;
                    # m = slope*s*ps + r on DVE (single PSUM read);
                    # y = m + x on DVE (all-fp16 SBUF: fast mode)
                    r4 = tpool.tile([P, GROUP_ROWS, W], FP16, name="r")
                    nc.scalar.activation(
                        r4, ps4, AF.Relu,
                        bias=pt[:, PBB:PBB + 1], scale=pt[:, PA:PA + 1],
                    )
                    m4 = tpool.tile([P, GROUP_ROWS, W], FP16, name="m")
                    nc.vector.scalar_tensor_tensor(
                        m4, ps4, pt[:, PD:PD + 1], r4, ALU.mult, ALU.add
                    )
                    nc.vector.tensor_tensor(u4, m4, x4, ALU.add)
                    if with_ec:
                        nc.gpsimd.tensor_scalar(u4, u4, pt[:, PEc:PEc + 1],
                                                None, ALU.add)
                else:
                    # v = ps*s + b0; m = min(v,0)*(slope-1); u = v + m;
                    # y = (u + b1) + x
                    v4 = tpool.tile([P, GROUP_ROWS, W], F32, name="v")
                    nc.vector.tensor_scalar(
                        v4, ps4, pt[:, PS:PS + 1], pt[:, PB0:PB0 + 1],
                        ALU.mult, ALU.add,
                    )
                    m4 = tpool.tile([P, GROUP_ROWS, W], F32, name="mw")
                    nc.vector.tensor_scalar(
                        m4, v4, 0.0, pt[:, PCM:PCM + 1], ALU.min, ALU.mult
                    )
                    nc.vector.tensor_tensor(v4, v4, m4, ALU.add)
                    nc.vector.scalar_tensor_tensor(
                        u4, v4, pt[:, PB1:PB1 + 1], x4, ALU.add, ALU.add
                    )
                # last strip: eager stores every 2 groups shorten the drain
                if s == NST - 1 and g % 2 == 1:
                    r0s = (g - 1) * GROUP_ROWS
                    nc.scalar.dma_start(
                        out=y3[:, h0 + r0s:h0 + r + GROUP_ROWS, :],
                        in_=ys[:, r0s:r + GROUP_ROWS, :])
            if s == NST - 1:
                if NG % 2 == 1:
                    r0s = (NG - 1) * GROUP_ROWS
                    nc.scalar.dma_start(out=y3[:, h0 + r0s:h0 + HS_S, :],
                                        in_=ys[:, r0s:HS_S, :])
            else:
                for q in range(2):
                    rq = q * (HS_S // 2)
                    rq1 = (q + 1) * (HS_S // 2)
                    nc.scalar.dma_start(out=y3[:, h0 + rq:h0 + rq1, :],
                                        in_=ys[:, rq:rq1, :])
            if s + 1 < NST:
                sign_strip(s + 1, *nxt, NEXT_CHUNKS[2:],
                           skip=sum(NEXT_CHUNKS[:2]))
            cur = nxt


def build_nc(zero_bias=True, with_ec=False):
    nc = bacc.Bacc("TRN2", target_bir_lowering=False, debug=False,
                   num_devices=NCORES)
    x_d = nc.dram_tensor("xin", [P, H * W], FP16, kind="ExternalInput").ap()
    w_d = nc.dram_tensor("wp", [P, 9 * 128], FP8, kind="ExternalInput").ap()
    p_d = nc.dram_tensor("pp", [P, NPAR], F32, kind="ExternalInput").ap()
    y_d = nc.dram_tensor("yout", [P, H * W], FP16, kind="ExternalOutput").ap()
    with tile.TileContext(nc) as tc:
        _emit(tc, nc, x_d, w_d, p_d, y_d, zero_bias, with_ec)
    nc.compile()
    return nc


_NC_CACHE = {}


def _get_nc(zero_bias, with_ec=False):
    key = (zero_bias, with_ec)
    if key not in _NC_CACHE:
        _NC_CACHE[key] = build_nc(zero_bias, with_ec)
    return _NC_CACHE[key]


def make_inputs(x, rd_k, rd_b, beta, conv_w, pr_bias0, prelu_w, pr_bias1):
    """Host-side prep: per-channel param table, packed sign weights, shards."""
    k = np.asarray(rd_k, np.float32).reshape(C)
    b = np.asarray(rd_b, np.float32).reshape(C)
    s = np.mean(np.abs(np.asarray(conv_w, np.float32)), axis=(1, 2, 3))
    b0 = np.asarray(pr_bias0, np.float32).reshape(C)
    slope = np.asarray(prelu_w, np.float32).reshape(C)
    b1 = np.asarray(pr_bias1, np.float32).reshape(C)
    cm = slope - 1.0
    cols = np.stack([
        k, b,
        (1.0 - slope) * s, (1.0 - slope) * b0,          # PA, PBB
        slope * s, slope * b0 + b1,                     # PD, PEc
        s, b0, cm, b1,                                  # fallback
        np.zeros(C, np.float32), np.zeros(C, np.float32),
    ], axis=1)
    pp = np.concatenate([cols, cols], axis=0).astype(np.float32)  # [128, 12]

    sw = np.sign(np.asarray(conv_w, np.float32))  # [co, ci, kh, kw]

    def blockdiag(kh, kw):
        S = sw[:, :, kh, kw].T  # [ci, co]
        out = np.zeros((P, P), np.float32)
        out[0:C, 0:C] = S
        out[C:P, C:P] = S
        return out

    wp = np.zeros((P, 9, 128), np.float32)
    for j in range(3):
        for d in range(2):
            wp[:, 2 * j + d, :] = blockdiag(j, d)
    for d in range(2):
        wp[:, 6 + d, :] = blockdiag(d, 2)
    wp[:, 8, :] = blockdiag(2, 2)
    wp = np.ascontiguousarray(wp.reshape(P, 9 * 128)).astype(mybir.dt.np(FP8))

    x = np.asarray(x, np.float32)
    in_maps = []
    for c in range(NCORES):
        xc = np.ascontiguousarray(x[2 * c:2 * c + 2]).reshape(P, H * W)
        in_maps.append({"xin": xc.astype(np.float16), "wp": wp, "pp": pp})
    return in_maps


def kernel(x, rd_k, rd_b, beta, conv_w, pr_bias0, prelu_w, pr_bias1):
    slope = np.asarray(prelu_w, np.float32).reshape(C)
    # relu decomposition of prelu needs (1-slope) >= 0
    zero_bias = bool(np.all((slope >= 0.0) & (slope <= 1.0)))
    ec = (slope * np.asarray(pr_bias0, np.float32).reshape(C)
          + np.asarray(pr_bias1, np.float32).reshape(C))
    with_ec = bool(np.any(ec != 0.0))
    in_maps = make_inputs(x, rd_k, rd_b, beta, conv_w, pr_bias0, prelu_w,
                          pr_bias1)
    nc = _get_nc(zero_bias, with_ec)
    res = run_bass_kernel_spmd(nc, in_maps, core_ids=list(range(NCORES)))
    y = np.empty((B, C, H, W), np.float32)
    for c in range(NCORES):
        y[2 * c:2 * c + 2] = (
            res.results[c]["yout"].astype(np.float32).reshape(2, C, H, W))
    return y
